# revision 34
# baseline (speedup 1.0000x reference)
"""Trainium2 Bass kernel for temporal-window GNN mean aggregation.

    out = x + scatter_mean(x[src] * mask, dst),
    mask = (edge_time <= seed_time[dst]) & (edge_time > seed_time[dst] - 100)

Sharding: destination-node sharding across 8 cores (no collectives), with
dst nodes assigned to 128-wide windows in seed_time-sorted order so each
window spans only ~1-2 distinct seed times.  Host work is layout only:
sort nodes by seed_time, sort edges into per-(window, src-bank) slot
grids restricted to each window's conservative candidate time range
(a superset of any possible masked edge for that window), and ship
per-slot metadata (edge_time, seed_time[dst], dst-local index).  All
reference arithmetic — the exact temporal mask compare, the masked
segment sums / counts (one-hot matmul on the PE array), the divide and
the residual add — happens on device.

Device per core (SPMD, one program):
  phase 0: DVE computes the exact mask m per slot and folds it into the
           one-hot key (no iota match -> slot contributes 0).
  chunk loop (4 chunks of 24-26 windows, pipelined via tile pools):
    - 4x dma_gather on 4 SWDGE queues (one per 25089-row src bank, int16
      index limit) fetch 512-byte x16 rows = [128 features, 1.0, pad]
      for the chunk's slots, bank-major into scratch; pad slots point at
      spread-out rows (same-address pads serialize the DMA engines)
    - 16 contiguous SBUF->SBUF DMA copies (split across the SP and ACT
      HWDGE queues) re-lay scratch into the window-major grid (32 slots
      per bank per window = 128 slots/window, grid columns in w%4-major
      order so both copy sides are plain 2D rectangles)
    - per window: one [128x128]@[128x129] matmul accumulates
      PSUM = S^T @ G (S = one-hot of the masked key, built 8 windows per
      DVE op); rare overflow slots (>32 candidates per window-bank) ride
      in a tail region of chunk 0's gather and add one extra matmul
      each; 3 windows share a PSUM bank so count/max/reciprocal are
      batched
    - mean via ACT scale / DVE tensor_scalar (alternating windows),
      residual add batched per chunk, f16 store per chunk
"""

import sys

import numpy as np

for _p in ("/opt/trn_rl_repo",):
    if _p not in sys.path:
        sys.path.insert(0, _p)

import concourse.bass as bass
import concourse.mybir as mybir
import concourse.tile as tile
from concourse import bacc
from concourse.bass_utils import run_bass_kernel_spmd

P = 128            # SBUF partitions == dst-window size
D = 128            # feature dim
NCORES = 8
W = 98             # dst windows per core
NODES_PC = W * P   # 12544 dst nodes per core
NPAD = NCORES * NODES_PC  # 100352
TW = 100           # time window

NBANKS = 4         # int16 gather-index banks over x16 rows
BANK = 25089       # rows per bank (<= 32768), NBANKS*BANK >= N
XROWS = NBANKS * BANK
V = 32             # slots per (window, bank); V*NBANKS = 128 = one block
TAILCAP = 64       # overflow slots per (core, bank), in chunk 0's tail
HW_ = [24, 24, 24, 26]   # windows per chunk
HBASE = [0, 24, 48, 72]
# slots per (chunk, bank) call: c0: 24*32 + 64 tail + 64 pad = 896
# (7 cols, tail at col 6 parts 0:64); c1/c2: 768; c3: 26*32 -> 896
HNIDX = [896, 768, 768, 896]
HCOLS = [n // P for n in HNIDX]
NCHUNK = len(HW_)
TAILPOS = 24 * V   # 768: first tail slot position in chunk 0 calls
TAILCOL = TAILPOS // P  # 6
ICOL0 = np.cumsum([0] + [n // 16 for n in HNIDX for _ in range(NBANKS)])
ICOLS_TOT = int(ICOL0[-1])
MCOLS = 3 * W + 4 * NBANKS   # concatenated metadata columns

f32 = mybir.dt.float32
f16 = mybir.dt.float16
i32 = mybir.dt.int32
i16 = mybir.dt.int16
OP = mybir.AluOpType


def build_program(segs: tuple):
    """segs: sorted tuple of (window, bank) overflow segments (union over
    cores); cores without a given overflow see an all-invalid tail key
    there and the extra matmul adds zero."""
    nc = bacc.Bacc(
        "TRN2", target_bir_lowering=False, debug=False, num_devices=NCORES,
        num_swdge_queues=4,
    )

    x16 = nc.dram_tensor("x16", [XROWS, 2 * D], f16, kind="ExternalInput")
    xsh = nc.dram_tensor("xs", [NODES_PC, D], f16, kind="ExternalInput")
    idx16 = nc.dram_tensor("idx16", [P, ICOLS_TOT], i16, kind="ExternalInput")
    meta_in = nc.dram_tensor("meta", [P, MCOLS], f16, kind="ExternalInput")
    out = nc.dram_tensor("out", [NODES_PC, D], f16, kind="ExternalOutput")

    segs_by_w = {}
    for (sw, sj) in segs:
        segs_by_w.setdefault(sw, []).append(sj)

    with tile.TileContext(nc) as tc:
        with (
            tc.tile_pool(name="meta", bufs=1) as meta,
            tc.tile_pool(name="scr0", bufs=1) as scr0p,
            tc.tile_pool(name="scr", bufs=2) as scrp,
            tc.tile_pool(name="grid", bufs=2) as gridp,
            tc.tile_pool(name="sbuf_s", bufs=4) as sp,
            tc.tile_pool(name="oc", bufs=2) as oc,
            tc.tile_pool(name="small", bufs=6) as small,
            tc.tile_pool(name="psum", bufs=5, space="PSUM") as psum_tp,
        ):
            # ---------------- phase 0: metadata + keys ----------------
            meta_t = meta.tile([P, MCOLS], f16)
            idx_t = meta.tile([P, ICOLS_TOT], i16)
            nc.sync.dma_start(out=meta_t[:], in_=meta_in[:])
            nc.sync.dma_start(out=idx_t[:], in_=idx16[:])
            et_t = meta_t[:, 0: W]
            st_t = meta_t[:, W: 2 * W]
            dl_t = meta_t[:, 2 * W: 3 * W]
            ett_t = meta_t[:, 3 * W: 3 * W + NBANKS]
            stt_t = meta_t[:, 3 * W + NBANKS: 3 * W + 2 * NBANKS]
            dlt_t = meta_t[:, 3 * W + 2 * NBANKS: 3 * W + 3 * NBANKS]
            wnt_t = meta_t[:, 3 * W + 3 * NBANKS: 3 * W + 4 * NBANKS]

            # iota_f[p, d] = d
            iota_i = meta.tile([P, P], i32)
            nc.gpsimd.iota(iota_i[:], pattern=[[1, P]], base=0,
                           channel_multiplier=0)
            iota_f = meta.tile([P, P], f16)
            nc.vector.tensor_copy(out=iota_f[:], in_=iota_i[:])

            # grid keys: key = dl - 300*m, m = (st-et in [0, TW))
            # (dl stores dst_local + 300; pads store 1300)
            d_g = meta.tile([P, W], f16)
            nc.vector.tensor_tensor(out=d_g[:], in0=st_t, in1=et_t,
                                    op=OP.subtract)
            m1 = meta.tile([P, W], f16)
            nc.vector.tensor_scalar(out=m1[:], in0=d_g[:], scalar1=0.0,
                                    scalar2=None, op0=OP.is_ge)
            m2 = meta.tile([P, W], f16)
            nc.vector.tensor_scalar(out=m2[:], in0=d_g[:], scalar1=float(TW),
                                    scalar2=None, op0=OP.is_lt)
            m_g = meta.tile([P, W], f16)
            nc.vector.tensor_tensor(out=m_g[:], in0=m1[:], in1=m2[:],
                                    op=OP.mult)
            m300 = meta.tile([P, W], f16)
            nc.vector.tensor_scalar(out=m300[:], in0=m_g[:], scalar1=300.0,
                                    scalar2=None, op0=OP.mult)
            key_g = meta.tile([P, W], f16)
            nc.vector.tensor_tensor(out=key_g[:], in0=dl_t, in1=m300[:],
                                    op=OP.subtract)

            # tail keys per bank: kt4 = dl_tail - 300*m_tail + 400
            # (per segment, subtract 400*(win==w) so only that window's
            #  tail slots land in [0,128))
            d_q = meta.tile([P, NBANKS], f16)
            nc.vector.tensor_tensor(out=d_q[:], in0=stt_t, in1=ett_t,
                                    op=OP.subtract)
            tm1 = meta.tile([P, NBANKS], f16)
            nc.vector.tensor_scalar(out=tm1[:], in0=d_q[:], scalar1=0.0,
                                    scalar2=None, op0=OP.is_ge)
            tm2 = meta.tile([P, NBANKS], f16)
            nc.vector.tensor_scalar(out=tm2[:], in0=d_q[:], scalar1=float(TW),
                                    scalar2=None, op0=OP.is_lt)
            tm = meta.tile([P, NBANKS], f16)
            nc.vector.tensor_tensor(out=tm[:], in0=tm1[:], in1=tm2[:],
                                    op=OP.mult)
            tm300 = meta.tile([P, NBANKS], f16)
            nc.vector.tensor_scalar(out=tm300[:], in0=tm[:], scalar1=300.0,
                                    scalar2=None, op0=OP.mult)
            kt = meta.tile([P, NBANKS], f16)
            nc.vector.tensor_tensor(out=kt[:], in0=dlt_t, in1=tm300[:],
                                    op=OP.subtract)
            kt4 = meta.tile([P, NBANKS], f16)
            nc.vector.tensor_scalar(out=kt4[:], in0=kt[:], scalar1=400.0,
                                    scalar2=None, op0=OP.add)

            # ---------------- main loop over chunks ----------------
            scr0 = [None] * NBANKS
            for h in range(NCHUNK):
                nw = HW_[h]
                ncols = HCOLS[h]
                scr = [None] * NBANKS
                for j in range(NBANKS):
                    if h == 0:
                        t = scr0p.tile([P, ncols * 2 * D], f16,
                                       tag=f"s0b{j}", name=f"scr0_{j}")
                        scr0[j] = t
                    else:
                        t = scrp.tile([P, ncols * 2 * D], f16,
                                      tag=f"sb{j}", name=f"scr{h}_{j}")
                    scr[j] = t
                    icol0 = int(ICOL0[h * NBANKS + j])
                    icn = HNIDX[h] // 16
                    nc.gpsimd.dma_gather(
                        out_ap=t[:].rearrange("p (k c) -> p k c", c=2 * D),
                        in_ap=x16[j * BANK:, :],
                        idxs_ap=idx_t[:, icol0: icol0 + icn],
                        num_idxs=HNIDX[h],
                        num_idxs_reg=HNIDX[h],
                        elem_size=2 * D,
                        single_packet=False,
                        queue_num=j,
                    )

                # qq-major grid column order -> every copy is contiguous 2D
                qq_start = {}
                pos = 0
                for qq in range(4):
                    n_qq = len(range(qq, nw, 4))
                    qq_start[qq] = (pos, n_qq)
                    pos += n_qq
                col_of = {}
                for qq in range(4):
                    cst, n_qq = qq_start[qq]
                    for i, wl in enumerate(range(qq, nw, 4)):
                        col_of[wl] = cst + i

                g_t = gridp.tile([P, nw * 2 * D], f16, tag="g",
                                 name=f"grid{h}")
                for j in range(NBANKS):
                    for qq in range(4):
                        cst, n_qq = qq_start[qq]
                        eng = nc.sync if qq % 2 == 0 else nc.scalar
                        eng.dma_start(
                            out=g_t[32 * j: 32 * j + 32,
                                    cst * 2 * D: (cst + n_qq) * 2 * D],
                            in_=scr[j][32 * qq: 32 * qq + 32,
                                       0: n_qq * 2 * D],
                        )

                # residual rows (d-major host layout: row = d*W + w)
                x_t = oc.tile([P, nw * D], f16, tag="x", name=f"x_{h}")
                nc.sync.dma_start(
                    out=x_t[:].rearrange("p (w f) -> p w f", f=D),
                    in_=xsh[:].rearrange("(d w) f -> d w f", w=W)
                    [:, HBASE[h]: HBASE[h] + nw, :],
                )

                o_t = oc.tile([P, nw * D], f16, tag="o", name=f"o_{h}")

                # one-hot builds batched 8 windows per op
                s_ts = {}
                for bi in range(0, nw, 8):
                    bn = min(8, nw - bi)
                    s8 = sp.tile([P, bn * P], f16, tag="s",
                                 name=f"s8_{h}_{bi}")
                    nc.vector.tensor_tensor(
                        out=s8[:].rearrange("p (w c) -> p w c", c=P),
                        in0=iota_f[:].unsqueeze(1).to_broadcast([P, bn, P]),
                        in1=key_g[:, HBASE[h] + bi: HBASE[h] + bi + bn]
                        .unsqueeze(2).to_broadcast([P, bn, P]),
                        op=OP.is_equal,
                    )
                    for k in range(bn):
                        s_ts[bi + k] = s8[:, k * P: (k + 1) * P]

                # psum triples: 3 windows share one PSUM bank
                for ti in range(0, nw, 3):
                    tn = min(3, nw - ti)
                    ps = psum_tp.tile([P, tn * (D + 1)], f32, tag="ps",
                                      name=f"ps_{h}_{ti}")
                    for k in range(tn):
                        wl = ti + k
                        w = HBASE[h] + wl
                        tail_js = segs_by_w.get(w, [])
                        gc = col_of[wl]
                        nc.tensor.matmul(
                            out=ps[:, k * (D + 1): (k + 1) * (D + 1)],
                            lhsT=s_ts[wl],
                            rhs=g_t[:, gc * 2 * D: gc * 2 * D + D + 1],
                            start=True,
                            stop=(len(tail_js) == 0),
                        )
                        for si, sj in enumerate(tail_js):
                            # select this window's tail slots in bank sj
                            v_t = small.tile([P, 1], f16, tag="v")
                            nc.vector.tensor_scalar(
                                out=v_t[:], in0=wnt_t[:, sj: sj + 1],
                                scalar1=float(w), scalar2=400.0,
                                op0=OP.is_equal, op1=OP.mult,
                            )
                            kseg = small.tile([P, 1], f16, tag="k")
                            nc.vector.tensor_tensor(
                                out=kseg[:], in0=kt4[:, sj: sj + 1],
                                in1=v_t[:], op=OP.subtract,
                            )
                            st_s = sp.tile([P, P], f16, tag="st")
                            nc.vector.tensor_tensor(
                                out=st_s[:],
                                in0=iota_f[:],
                                in1=kseg[:].to_broadcast([P, P]),
                                op=OP.is_equal,
                            )
                            nc.tensor.matmul(
                                out=ps[:, k * (D + 1): (k + 1) * (D + 1)],
                                lhsT=st_s[:],
                                rhs=scr0[sj][:, TAILCOL * 2 * D:
                                             TAILCOL * 2 * D + D + 1],
                                start=False,
                                stop=(si == len(tail_js) - 1),
                            )

                    psv = ps[:].rearrange("p (w c) -> p w c", c=D + 1)
                    cnt_t = small.tile([P, 3], f32, tag="cnt")
                    nc.vector.tensor_scalar(out=cnt_t[:, 0:tn],
                                            in0=psv[:, :, D],
                                            scalar1=1.0, scalar2=None,
                                            op0=OP.max)
                    rcp_t = small.tile([P, 3], f32, tag="rcp")
                    nc.vector.reciprocal(out=rcp_t[:, 0:tn],
                                         in_=cnt_t[:, 0:tn])
                    for k in range(tn):
                        wl = ti + k
                        # mean-divide alternates between ACT and DVE to
                        # balance engine load
                        if wl % 2 == 0:
                            nc.scalar.activation(
                                out=o_t[:, wl * D: (wl + 1) * D],
                                in_=ps[:, k * (D + 1): k * (D + 1) + D],
                                func=mybir.ActivationFunctionType.Copy,
                                scale=rcp_t[:, k: k + 1],
                            )
                        else:
                            nc.vector.tensor_scalar(
                                out=o_t[:, wl * D: (wl + 1) * D],
                                in0=ps[:, k * (D + 1): k * (D + 1) + D],
                                scalar1=rcp_t[:, k: k + 1],
                                scalar2=None, op0=OP.mult,
                            )

                # residual add, one op per half
                nc.vector.tensor_tensor(out=o_t[:], in0=o_t[:], in1=x_t[:],
                                        op=OP.add)
                nc.sync.dma_start(
                    out=out[:].rearrange("(d w) f -> d w f", w=W)
                    [:, HBASE[h]: HBASE[h] + nw, :],
                    in_=o_t[:].rearrange("p (w f) -> p w f", f=D),
                )

    nc.compile()
    return nc


_PROGRAM_CACHE: dict[tuple, object] = {}


def _get_program(segs: tuple):
    if segs not in _PROGRAM_CACHE:
        _PROGRAM_CACHE[segs] = build_program(segs)
    return _PROGRAM_CACHE[segs]


def _prep_inputs(x, edge_index, edge_time, seed_time):
    """Host-side layout: st-sorted windows, conservative candidate slots,
    uniform V-grid + overflow tails, wrapped int16 gather-index planes."""
    x = np.asarray(x, dtype=np.float32)
    ei = np.asarray(edge_index)
    et = np.asarray(edge_time).astype(np.int64)
    st = np.asarray(seed_time).astype(np.int64)
    N = x.shape[0]
    assert N <= NPAD and N <= XROWS

    src = ei[0].astype(np.int64)
    dst = ei[1].astype(np.int64)

    order = np.argsort(st, kind="stable")
    newid = np.empty(N, np.int64)
    newid[order] = np.arange(N)

    st_pad = np.full(NPAD, -10**6, np.int64)
    st_pad[:N] = st[order]
    wins = st_pad.reshape(-1, P)
    has = (wins > -10**5).any(1)
    st_lo = np.where(has, np.where(wins > -10**5, wins, 10**9).min(1), 0)
    st_hi = np.where(has, np.where(wins > -10**5, wins, -10**9).max(1), -10**6)

    dst_new = newid[dst]
    g_e = dst_new >> 7
    cand = (et > st_lo[g_e] - TW) & (et <= st_hi[g_e])

    csrc = src[cand]
    cet = et[cand]
    cst = st[dst[cand]]
    cg = g_e[cand]
    cdl = dst_new[cand] % P
    cbank = csrc // BANK

    key2 = cg * NBANKS + cbank
    o2 = np.argsort(key2, kind="stable")
    binc = np.bincount(key2, minlength=NCORES * W * NBANKS)
    offs = np.zeros(NCORES * W * NBANKS, np.int64)
    np.cumsum(binc[:-1], out=offs[1:])
    rank = np.empty(len(o2), np.int64)
    rank[o2] = np.arange(len(o2)) - offs[key2[o2]]

    core_e = cg // W
    w_e = cg % W
    is_main = rank < V

    # grid metadata [NCORES, P, W]
    et_g = np.zeros((NCORES, P, W), np.float16)
    st_g = np.full((NCORES, P, W), -2000.0, np.float16)
    dl_g = np.full((NCORES, P, W), 1300.0, np.float16)
    mc, mp, mw = (core_e[is_main], cbank[is_main] * V + rank[is_main],
                  w_e[is_main])
    et_g[mc, mp, mw] = cet[is_main].astype(np.float16)
    st_g[mc, mp, mw] = cst[is_main].astype(np.float16)
    dl_g[mc, mp, mw] = cdl[is_main].astype(np.float16) + 300.0

    # overflow tails [NCORES, P, NBANKS] (partitions 0..TAILCAP-1 used)
    et_a = np.zeros((NCORES, P, NBANKS), np.float16)
    st_a = np.full((NCORES, P, NBANKS), -2000.0, np.float16)
    dl_a = np.full((NCORES, P, NBANKS), 1300.0, np.float16)
    wn_a = np.full((NCORES, P, NBANKS), -1.0, np.float16)

    okey = (core_e * NBANKS + cbank)[~is_main]
    oo = np.argsort(okey, kind="stable")
    obinc = np.bincount(okey, minlength=NCORES * NBANKS)
    assert obinc.max() <= TAILCAP, f"tail overflow: {obinc.max()}"
    ooffs = np.zeros(NCORES * NBANKS, np.int64)
    np.cumsum(obinc[:-1], out=ooffs[1:])
    t_pos = np.empty(len(oo), np.int64)
    t_pos[oo] = np.arange(len(oo)) - ooffs[okey[oo]]
    tc, tj = core_e[~is_main], cbank[~is_main]
    et_a[tc, t_pos, tj] = cet[~is_main].astype(np.float16)
    st_a[tc, t_pos, tj] = cst[~is_main].astype(np.float16)
    dl_a[tc, t_pos, tj] = cdl[~is_main].astype(np.float16) + 300.0
    wn_a[tc, t_pos, tj] = w_e[~is_main].astype(np.float16)

    segs = tuple(sorted(set(zip(w_e[~is_main].tolist(),
                                cbank[~is_main].tolist()))))

    # gather index planes, wrapped [16, n/16], replicated to 128 partitions.
    # Pad slots point at spread-out rows (not all row 0) to avoid
    # same-address hotspots in the DMA engines.
    pad_rows = (np.arange(16 * ICOLS_TOT, dtype=np.int64) * 97) % 24000
    idx_a = np.broadcast_to(
        pad_rows.reshape(16, ICOLS_TOT), (NCORES, 16, ICOLS_TOT)
    ).astype(np.int16).copy()
    h_of_w = np.zeros(W, np.int64)
    for hi in range(NCHUNK):
        h_of_w[HBASE[hi]: HBASE[hi] + HW_[hi]] = hi
    mh = h_of_w[mw]
    mpos = (mw - np.array(HBASE)[mh]) * V + (mp % V)
    micol = ICOL0[mh * NBANKS + cbank[is_main]] + mpos // 16
    idx_a[mc, mpos % 16, micol] = (csrc[is_main]
                                   - cbank[is_main] * BANK).astype(np.int16)
    # tail slots: half 0, positions TAILPOS + t
    tpos = TAILPOS + t_pos
    ticol = ICOL0[0 * NBANKS + tj] + tpos // 16
    idx_a[tc, tpos % 16, ticol] = (csrc[~is_main]
                                   - tj * BANK).astype(np.int16)
    idx_rep = np.tile(idx_a, (1, 8, 1))

    # concatenated metadata [P, 3W + 16]
    meta_all = np.concatenate(
        [et_g, st_g, dl_g, et_a, st_a, dl_a, wn_a], axis=2)

    x_pad = np.zeros((NPAD, D), np.float32)
    x_pad[:N] = x[order]
    x16 = np.zeros((XROWS, 2 * D), np.float16)
    x16[:N, :D] = x.astype(np.float16)
    x16[:, D] = 1.0
    x_shards = x_pad.reshape(NCORES, NODES_PC, D)

    in_maps = [
        {
            "x16": x16,
            # d-major layout: row = d*W + w, so per-partition DMA streams
            # are contiguous
            "xs": np.ascontiguousarray(
                x_shards[c].reshape(W, P, D).transpose(1, 0, 2)
                .reshape(NODES_PC, D)).astype(np.float16),
            "idx16": idx_rep[c],
            "meta": meta_all[c],
        }
        for c in range(NCORES)
    ]
    return in_maps, segs, order, N


def _postprocess(results, order, N):
    res = np.concatenate(
        [results[c]["out"].astype(np.float32).reshape(P, W, D)
         .transpose(1, 0, 2).reshape(NODES_PC, D) for c in range(NCORES)],
        axis=0,
    )
    out = np.empty((N, D), np.float32)
    out[order] = res[:N]
    return out


def kernel(x, edge_index, edge_time, seed_time):
    in_maps, segs, order, N = _prep_inputs(x, edge_index, edge_time,
                                           seed_time)
    nc = _get_program(segs)
    res = run_bass_kernel_spmd(nc, in_maps, core_ids=list(range(NCORES)))
    return _postprocess(res.results, order, N)


# revision 39
# speedup vs baseline: 1.0339x; 1.0339x over previous
"""Trainium2 Bass kernel for temporal-window GNN mean aggregation.

    out = x + scatter_mean(x[src] * mask, dst),
    mask = (edge_time <= seed_time[dst]) & (edge_time > seed_time[dst] - 100)

Sharding: destination-node sharding across 8 cores (no collectives), with
dst nodes assigned to 128-wide windows in seed_time-sorted order so each
window spans only ~1-2 distinct seed times.  Host work is layout only:
sort nodes by seed_time, sort edges into per-(window, src-bank) slot
grids restricted to each window's conservative candidate time range
(a superset of any possible masked edge for that window), and ship
per-slot metadata (edge_time, seed_time[dst], dst-local index).  All
reference arithmetic — the exact temporal mask compare, the masked
segment sums / counts (one-hot matmul on the PE array), the divide and
the residual add — happens on device.

Device per core (SPMD, one program):
  phase 0: DVE computes the exact mask m per slot and folds it into the
           one-hot key (no iota match -> slot contributes 0).
  chunk loop (4 chunks of 24-26 windows, pipelined via tile pools):
    - 4x dma_gather on 4 SWDGE queues (one per 25089-row src bank, int16
      index limit) fetch 512-byte x16 rows = [128 features, 1.0, pad]
      for the chunk's slots, bank-major into scratch; pad slots point at
      spread-out rows (same-address pads serialize the DMA engines)
    - 16 contiguous SBUF->SBUF DMA copies (split across the SP and ACT
      HWDGE queues) re-lay scratch into the window-major grid (32 slots
      per bank per window = 128 slots/window, grid columns in w%4-major
      order so both copy sides are plain 2D rectangles)
    - per window: one [128x128]@[128x129] matmul accumulates
      PSUM = S^T @ G (S = one-hot of the masked key, built 8 windows per
      DVE op); rare overflow slots (>32 candidates per window-bank) ride
      in a tail region of chunk 0's gather and add one extra matmul
      each; 3 windows share a PSUM bank so count/max/reciprocal are
      batched
    - mean via ACT scale / DVE tensor_scalar (alternating windows),
      residual add batched per chunk, f16 store per chunk
"""

import sys

import numpy as np

for _p in ("/opt/trn_rl_repo",):
    if _p not in sys.path:
        sys.path.insert(0, _p)

import concourse.bass as bass
import concourse.mybir as mybir
import concourse.tile as tile
from concourse import bacc
from concourse.bass_utils import run_bass_kernel_spmd

P = 128            # SBUF partitions == dst-window size
D = 128            # feature dim
NCORES = 8
W = 98             # dst windows per core
NODES_PC = W * P   # 12544 dst nodes per core
NPAD = NCORES * NODES_PC  # 100352
TW = 100           # time window

NBANKS = 4         # int16 gather-index banks over x16 rows
BANK = 25089       # rows per bank (<= 32768), NBANKS*BANK >= N
XROWS = NBANKS * BANK
V = 32             # slots per (window, bank); V*NBANKS = 128 = one block
TAILCAP = 64       # overflow slots per (core, bank), in chunk 0's tail
HW_ = [24, 24, 24, 24, 2]   # windows per chunk; tiny last chunk so the
HBASE = [0, 24, 48, 72, 96]  # end-of-program compute tail is short
# slots per (chunk, bank) call: c0: 24*32 + 64 tail + 64 pad = 896
# (7 cols, tail at col 6 parts 0:64); c1-c3: 768; c4: 2*32 -> 128
HNIDX = [896, 768, 768, 768, 128]
HCOLS = [n // P for n in HNIDX]
NCHUNK = len(HW_)
TAILPOS = 24 * V   # 768: first tail slot position in chunk 0 calls
TAILCOL = TAILPOS // P  # 6
ICOL0 = np.cumsum([0] + [n // 16 for n in HNIDX for _ in range(NBANKS)])
ICOLS_TOT = int(ICOL0[-1])
MCOLS = 3 * W + 4 * NBANKS   # concatenated metadata columns

f32 = mybir.dt.float32
f16 = mybir.dt.float16
i32 = mybir.dt.int32
i16 = mybir.dt.int16
OP = mybir.AluOpType


def build_program(segs: tuple):
    """segs: sorted tuple of (window, bank) overflow segments (union over
    cores); cores without a given overflow see an all-invalid tail key
    there and the extra matmul adds zero."""
    nc = bacc.Bacc(
        "TRN2", target_bir_lowering=False, debug=False, num_devices=NCORES,
        num_swdge_queues=4,
    )

    x16 = nc.dram_tensor("x16", [XROWS, 2 * D], f16, kind="ExternalInput")
    xsh = nc.dram_tensor("xs", [NODES_PC, D], f16, kind="ExternalInput")
    idx16 = nc.dram_tensor("idx16", [P, ICOLS_TOT], i16, kind="ExternalInput")
    meta_in = nc.dram_tensor("meta", [P, MCOLS], f16, kind="ExternalInput")
    out = nc.dram_tensor("out", [NODES_PC, D], f16, kind="ExternalOutput")

    segs_by_w = {}
    for (sw, sj) in segs:
        segs_by_w.setdefault(sw, []).append(sj)

    with tile.TileContext(nc) as tc:
        with (
            tc.tile_pool(name="meta", bufs=1) as meta,
            tc.tile_pool(name="scr0", bufs=1) as scr0p,
            tc.tile_pool(name="scr", bufs=2) as scrp,
            tc.tile_pool(name="grid", bufs=2) as gridp,
            tc.tile_pool(name="sbuf_s", bufs=4) as sp,
            tc.tile_pool(name="oc", bufs=2) as oc,
            tc.tile_pool(name="small", bufs=6) as small,
            tc.tile_pool(name="psum", bufs=5, space="PSUM") as psum_tp,
        ):
            # ---------------- phase 0: metadata + keys ----------------
            meta_t = meta.tile([P, MCOLS], f16)
            idx_t = meta.tile([P, ICOLS_TOT], i16)
            nc.sync.dma_start(out=idx_t[:], in_=idx16[:])
            nc.sync.dma_start(out=meta_t[:], in_=meta_in[:])

            # tiny warmup gather: absorbs the SWDGE cold-start (~7us)
            # before the first real call
            idx_d = meta.tile([P, 1], i16)
            nc.vector.memset(idx_d[:], 0)
            dummy = meta.tile([P, 2 * D], f16)
            nc.gpsimd.dma_gather(
                out_ap=dummy[:].rearrange("p (k c) -> p k c", c=2 * D),
                in_ap=x16[:, :],
                idxs_ap=idx_d[:],
                num_idxs=16,
                num_idxs_reg=16,
                elem_size=2 * D,
                single_packet=False,
                queue_num=0,
            )
            et_t = meta_t[:, 0: W]
            st_t = meta_t[:, W: 2 * W]
            dl_t = meta_t[:, 2 * W: 3 * W]
            ett_t = meta_t[:, 3 * W: 3 * W + NBANKS]
            stt_t = meta_t[:, 3 * W + NBANKS: 3 * W + 2 * NBANKS]
            dlt_t = meta_t[:, 3 * W + 2 * NBANKS: 3 * W + 3 * NBANKS]
            wnt_t = meta_t[:, 3 * W + 3 * NBANKS: 3 * W + 4 * NBANKS]

            # grid keys: key = dl - 300*m, m = (st-et in [0, TW))
            # (dl stores dst_local + 300; pads store 1300)
            d_g = meta.tile([P, W], f16)
            nc.vector.tensor_tensor(out=d_g[:], in0=st_t, in1=et_t,
                                    op=OP.subtract)
            m1 = meta.tile([P, W], f16)
            nc.vector.tensor_scalar(out=m1[:], in0=d_g[:], scalar1=0.0,
                                    scalar2=None, op0=OP.is_ge)
            m2 = meta.tile([P, W], f16)
            nc.vector.tensor_scalar(out=m2[:], in0=d_g[:], scalar1=float(TW),
                                    scalar2=None, op0=OP.is_lt)
            m_g = meta.tile([P, W], f16)
            nc.vector.tensor_tensor(out=m_g[:], in0=m1[:], in1=m2[:],
                                    op=OP.mult)
            m300 = meta.tile([P, W], f16)
            nc.vector.tensor_scalar(out=m300[:], in0=m_g[:], scalar1=300.0,
                                    scalar2=None, op0=OP.mult)
            key_g = meta.tile([P, W], f16)
            nc.vector.tensor_tensor(out=key_g[:], in0=dl_t, in1=m300[:],
                                    op=OP.subtract)

            # tail keys per bank: kt4 = dl_tail - 300*m_tail + 400
            # (per segment, subtract 400*(win==w) so only that window's
            #  tail slots land in [0,128))
            d_q = meta.tile([P, NBANKS], f16)
            nc.vector.tensor_tensor(out=d_q[:], in0=stt_t, in1=ett_t,
                                    op=OP.subtract)
            tm1 = meta.tile([P, NBANKS], f16)
            nc.vector.tensor_scalar(out=tm1[:], in0=d_q[:], scalar1=0.0,
                                    scalar2=None, op0=OP.is_ge)
            tm2 = meta.tile([P, NBANKS], f16)
            nc.vector.tensor_scalar(out=tm2[:], in0=d_q[:], scalar1=float(TW),
                                    scalar2=None, op0=OP.is_lt)
            tm = meta.tile([P, NBANKS], f16)
            nc.vector.tensor_tensor(out=tm[:], in0=tm1[:], in1=tm2[:],
                                    op=OP.mult)
            tm300 = meta.tile([P, NBANKS], f16)
            nc.vector.tensor_scalar(out=tm300[:], in0=tm[:], scalar1=300.0,
                                    scalar2=None, op0=OP.mult)
            kt = meta.tile([P, NBANKS], f16)
            nc.vector.tensor_tensor(out=kt[:], in0=dlt_t, in1=tm300[:],
                                    op=OP.subtract)
            kt4 = meta.tile([P, NBANKS], f16)
            nc.vector.tensor_scalar(out=kt4[:], in0=kt[:], scalar1=400.0,
                                    scalar2=None, op0=OP.add)

            # ---------------- main loop over chunks ----------------
            scr0 = [None] * NBANKS
            for h in range(NCHUNK):
                nw = HW_[h]
                ncols = HCOLS[h]
                scr = [None] * NBANKS
                for j in range(NBANKS):
                    if h == 0:
                        t = scr0p.tile([P, ncols * 2 * D], f16,
                                       tag=f"s0b{j}", name=f"scr0_{j}")
                        scr0[j] = t
                    else:
                        t = scrp.tile([P, ncols * 2 * D], f16,
                                      tag=f"sb{j}", name=f"scr{h}_{j}")
                    scr[j] = t
                    icol0 = int(ICOL0[h * NBANKS + j])
                    icn = HNIDX[h] // 16
                    nc.gpsimd.dma_gather(
                        out_ap=t[:].rearrange("p (k c) -> p k c", c=2 * D),
                        in_ap=x16[j * BANK:, :],
                        idxs_ap=idx_t[:, icol0: icol0 + icn],
                        num_idxs=HNIDX[h],
                        num_idxs_reg=HNIDX[h],
                        elem_size=2 * D,
                        single_packet=False,
                        queue_num=j,
                    )

                if h == 0:
                    # iota_f[p, d] = d (emitted after chunk 0's gathers so
                    # the Pool queue starts dispatching them immediately)
                    iota_i = meta.tile([P, P], i32)
                    nc.gpsimd.iota(iota_i[:], pattern=[[1, P]], base=0,
                                   channel_multiplier=0)
                    iota_f = meta.tile([P, P], f16)
                    nc.vector.tensor_copy(out=iota_f[:], in_=iota_i[:])

                # qq-major grid column order -> every copy is contiguous 2D
                qq_start = {}
                pos = 0
                for qq in range(4):
                    n_qq = len(range(qq, nw, 4))
                    qq_start[qq] = (pos, n_qq)
                    pos += n_qq
                col_of = {}
                for qq in range(4):
                    cst, n_qq = qq_start[qq]
                    for i, wl in enumerate(range(qq, nw, 4)):
                        col_of[wl] = cst + i

                g_t = gridp.tile([P, nw * 2 * D], f16, tag="g",
                                 name=f"grid{h}")
                for j in range(NBANKS):
                    for qq in range(4):
                        cst, n_qq = qq_start[qq]
                        if n_qq == 0:
                            continue
                        eng = nc.sync if qq % 2 == 0 else nc.scalar
                        eng.dma_start(
                            out=g_t[32 * j: 32 * j + 32,
                                    cst * 2 * D: (cst + n_qq) * 2 * D],
                            in_=scr[j][32 * qq: 32 * qq + 32,
                                       0: n_qq * 2 * D],
                        )

                # residual rows (d-major host layout: row = d*W + w)
                x_t = oc.tile([P, nw * D], f16, tag="x", name=f"x_{h}")
                nc.sync.dma_start(
                    out=x_t[:].rearrange("p (w f) -> p w f", f=D),
                    in_=xsh[:].rearrange("(d w) f -> d w f", w=W)
                    [:, HBASE[h]: HBASE[h] + nw, :],
                )

                o_t = oc.tile([P, nw * D], f16, tag="o", name=f"o_{h}")

                # one-hot builds batched 8 windows per op
                s_ts = {}
                for bi in range(0, nw, 8):
                    bn = min(8, nw - bi)
                    s8 = sp.tile([P, bn * P], f16, tag="s",
                                 name=f"s8_{h}_{bi}")
                    nc.vector.tensor_tensor(
                        out=s8[:].rearrange("p (w c) -> p w c", c=P),
                        in0=iota_f[:].unsqueeze(1).to_broadcast([P, bn, P]),
                        in1=key_g[:, HBASE[h] + bi: HBASE[h] + bi + bn]
                        .unsqueeze(2).to_broadcast([P, bn, P]),
                        op=OP.is_equal,
                    )
                    for k in range(bn):
                        s_ts[bi + k] = s8[:, k * P: (k + 1) * P]

                # psum triples: 3 windows share one PSUM bank
                for ti in range(0, nw, 3):
                    tn = min(3, nw - ti)
                    ps = psum_tp.tile([P, tn * (D + 1)], f32, tag="ps",
                                      name=f"ps_{h}_{ti}")
                    for k in range(tn):
                        wl = ti + k
                        w = HBASE[h] + wl
                        tail_js = segs_by_w.get(w, [])
                        gc = col_of[wl]
                        nc.tensor.matmul(
                            out=ps[:, k * (D + 1): (k + 1) * (D + 1)],
                            lhsT=s_ts[wl],
                            rhs=g_t[:, gc * 2 * D: gc * 2 * D + D + 1],
                            start=True,
                            stop=(len(tail_js) == 0),
                        )
                        for si, sj in enumerate(tail_js):
                            # select this window's tail slots in bank sj
                            v_t = small.tile([P, 1], f16, tag="v")
                            nc.vector.tensor_scalar(
                                out=v_t[:], in0=wnt_t[:, sj: sj + 1],
                                scalar1=float(w), scalar2=400.0,
                                op0=OP.is_equal, op1=OP.mult,
                            )
                            kseg = small.tile([P, 1], f16, tag="k")
                            nc.vector.tensor_tensor(
                                out=kseg[:], in0=kt4[:, sj: sj + 1],
                                in1=v_t[:], op=OP.subtract,
                            )
                            st_s = sp.tile([P, P], f16, tag="st")
                            nc.vector.tensor_tensor(
                                out=st_s[:],
                                in0=iota_f[:],
                                in1=kseg[:].to_broadcast([P, P]),
                                op=OP.is_equal,
                            )
                            nc.tensor.matmul(
                                out=ps[:, k * (D + 1): (k + 1) * (D + 1)],
                                lhsT=st_s[:],
                                rhs=scr0[sj][:, TAILCOL * 2 * D:
                                             TAILCOL * 2 * D + D + 1],
                                start=False,
                                stop=(si == len(tail_js) - 1),
                            )

                    psv = ps[:].rearrange("p (w c) -> p w c", c=D + 1)
                    cnt_t = small.tile([P, 3], f32, tag="cnt")
                    nc.vector.tensor_scalar(out=cnt_t[:, 0:tn],
                                            in0=psv[:, :, D],
                                            scalar1=1.0, scalar2=None,
                                            op0=OP.max)
                    rcp_t = small.tile([P, 3], f32, tag="rcp")
                    nc.vector.reciprocal(out=rcp_t[:, 0:tn],
                                         in_=cnt_t[:, 0:tn])
                    for k in range(tn):
                        wl = ti + k
                        # mean-divide alternates between ACT and DVE to
                        # balance engine load
                        if wl % 2 == 0:
                            nc.scalar.activation(
                                out=o_t[:, wl * D: (wl + 1) * D],
                                in_=ps[:, k * (D + 1): k * (D + 1) + D],
                                func=mybir.ActivationFunctionType.Copy,
                                scale=rcp_t[:, k: k + 1],
                            )
                        else:
                            nc.vector.tensor_scalar(
                                out=o_t[:, wl * D: (wl + 1) * D],
                                in0=ps[:, k * (D + 1): k * (D + 1) + D],
                                scalar1=rcp_t[:, k: k + 1],
                                scalar2=None, op0=OP.mult,
                            )

                # residual add, one op per half
                nc.vector.tensor_tensor(out=o_t[:], in0=o_t[:], in1=x_t[:],
                                        op=OP.add)
                nc.sync.dma_start(
                    out=out[:].rearrange("(d w) f -> d w f", w=W)
                    [:, HBASE[h]: HBASE[h] + nw, :],
                    in_=o_t[:].rearrange("p (w f) -> p w f", f=D),
                )

    nc.compile()
    return nc


_PROGRAM_CACHE: dict[tuple, object] = {}


def _get_program(segs: tuple):
    if segs not in _PROGRAM_CACHE:
        _PROGRAM_CACHE[segs] = build_program(segs)
    return _PROGRAM_CACHE[segs]


def _prep_inputs(x, edge_index, edge_time, seed_time):
    """Host-side layout: st-sorted windows, conservative candidate slots,
    uniform V-grid + overflow tails, wrapped int16 gather-index planes."""
    x = np.asarray(x, dtype=np.float32)
    ei = np.asarray(edge_index)
    et = np.asarray(edge_time).astype(np.int64)
    st = np.asarray(seed_time).astype(np.int64)
    N = x.shape[0]
    assert N <= NPAD and N <= XROWS

    src = ei[0].astype(np.int64)
    dst = ei[1].astype(np.int64)

    order = np.argsort(st, kind="stable")
    newid = np.empty(N, np.int64)
    newid[order] = np.arange(N)

    st_pad = np.full(NPAD, -10**6, np.int64)
    st_pad[:N] = st[order]
    wins = st_pad.reshape(-1, P)
    has = (wins > -10**5).any(1)
    st_lo = np.where(has, np.where(wins > -10**5, wins, 10**9).min(1), 0)
    st_hi = np.where(has, np.where(wins > -10**5, wins, -10**9).max(1), -10**6)

    dst_new = newid[dst]
    g_e = dst_new >> 7
    cand = (et > st_lo[g_e] - TW) & (et <= st_hi[g_e])

    csrc = src[cand]
    cet = et[cand]
    cst = st[dst[cand]]
    cg = g_e[cand]
    cdl = dst_new[cand] % P
    cbank = csrc // BANK

    key2 = cg * NBANKS + cbank
    o2 = np.argsort(key2, kind="stable")
    binc = np.bincount(key2, minlength=NCORES * W * NBANKS)
    offs = np.zeros(NCORES * W * NBANKS, np.int64)
    np.cumsum(binc[:-1], out=offs[1:])
    rank = np.empty(len(o2), np.int64)
    rank[o2] = np.arange(len(o2)) - offs[key2[o2]]

    core_e = cg // W
    w_e = cg % W
    is_main = rank < V

    # grid metadata [NCORES, P, W]
    et_g = np.zeros((NCORES, P, W), np.float16)
    st_g = np.full((NCORES, P, W), -2000.0, np.float16)
    dl_g = np.full((NCORES, P, W), 1300.0, np.float16)
    mc, mp, mw = (core_e[is_main], cbank[is_main] * V + rank[is_main],
                  w_e[is_main])
    et_g[mc, mp, mw] = cet[is_main].astype(np.float16)
    st_g[mc, mp, mw] = cst[is_main].astype(np.float16)
    dl_g[mc, mp, mw] = cdl[is_main].astype(np.float16) + 300.0

    # overflow tails [NCORES, P, NBANKS] (partitions 0..TAILCAP-1 used)
    et_a = np.zeros((NCORES, P, NBANKS), np.float16)
    st_a = np.full((NCORES, P, NBANKS), -2000.0, np.float16)
    dl_a = np.full((NCORES, P, NBANKS), 1300.0, np.float16)
    wn_a = np.full((NCORES, P, NBANKS), -1.0, np.float16)

    okey = (core_e * NBANKS + cbank)[~is_main]
    oo = np.argsort(okey, kind="stable")
    obinc = np.bincount(okey, minlength=NCORES * NBANKS)
    assert obinc.max() <= TAILCAP, f"tail overflow: {obinc.max()}"
    ooffs = np.zeros(NCORES * NBANKS, np.int64)
    np.cumsum(obinc[:-1], out=ooffs[1:])
    t_pos = np.empty(len(oo), np.int64)
    t_pos[oo] = np.arange(len(oo)) - ooffs[okey[oo]]
    tc, tj = core_e[~is_main], cbank[~is_main]
    et_a[tc, t_pos, tj] = cet[~is_main].astype(np.float16)
    st_a[tc, t_pos, tj] = cst[~is_main].astype(np.float16)
    dl_a[tc, t_pos, tj] = cdl[~is_main].astype(np.float16) + 300.0
    wn_a[tc, t_pos, tj] = w_e[~is_main].astype(np.float16)

    segs = tuple(sorted(set(zip(w_e[~is_main].tolist(),
                                cbank[~is_main].tolist()))))

    # gather index planes, wrapped [16, n/16], replicated to 128 partitions.
    # Pad slots point at spread-out rows (not all row 0) to avoid
    # same-address hotspots in the DMA engines.
    pad_rows = (np.arange(16 * ICOLS_TOT, dtype=np.int64) * 97) % 24000
    idx_a = np.broadcast_to(
        pad_rows.reshape(16, ICOLS_TOT), (NCORES, 16, ICOLS_TOT)
    ).astype(np.int16).copy()
    h_of_w = np.zeros(W, np.int64)
    for hi in range(NCHUNK):
        h_of_w[HBASE[hi]: HBASE[hi] + HW_[hi]] = hi
    mh = h_of_w[mw]
    mpos = (mw - np.array(HBASE)[mh]) * V + (mp % V)
    micol = ICOL0[mh * NBANKS + cbank[is_main]] + mpos // 16
    idx_a[mc, mpos % 16, micol] = (csrc[is_main]
                                   - cbank[is_main] * BANK).astype(np.int16)
    # tail slots: half 0, positions TAILPOS + t
    tpos = TAILPOS + t_pos
    ticol = ICOL0[0 * NBANKS + tj] + tpos // 16
    idx_a[tc, tpos % 16, ticol] = (csrc[~is_main]
                                   - tj * BANK).astype(np.int16)
    idx_rep = np.tile(idx_a, (1, 8, 1))

    # concatenated metadata [P, 3W + 16]
    meta_all = np.concatenate(
        [et_g, st_g, dl_g, et_a, st_a, dl_a, wn_a], axis=2)

    x_pad = np.zeros((NPAD, D), np.float32)
    x_pad[:N] = x[order]
    x16 = np.zeros((XROWS, 2 * D), np.float16)
    x16[:N, :D] = x.astype(np.float16)
    x16[:, D] = 1.0
    x_shards = x_pad.reshape(NCORES, NODES_PC, D)

    in_maps = [
        {
            "x16": x16,
            # d-major layout: row = d*W + w, so per-partition DMA streams
            # are contiguous
            "xs": np.ascontiguousarray(
                x_shards[c].reshape(W, P, D).transpose(1, 0, 2)
                .reshape(NODES_PC, D)).astype(np.float16),
            "idx16": idx_rep[c],
            "meta": meta_all[c],
        }
        for c in range(NCORES)
    ]
    return in_maps, segs, order, N


def _postprocess(results, order, N):
    res = np.concatenate(
        [results[c]["out"].astype(np.float32).reshape(P, W, D)
         .transpose(1, 0, 2).reshape(NODES_PC, D) for c in range(NCORES)],
        axis=0,
    )
    out = np.empty((N, D), np.float32)
    out[order] = res[:N]
    return out


def kernel(x, edge_index, edge_time, seed_time):
    in_maps, segs, order, N = _prep_inputs(x, edge_index, edge_time,
                                           seed_time)
    nc = _get_program(segs)
    res = run_bass_kernel_spmd(nc, in_maps, core_ids=list(range(NCORES)))
    return _postprocess(res.results, order, N)
